# revision 1
# baseline (speedup 1.0000x reference)
"""PNAConv message-passing kernel for 8 TRN2 NeuronCores.

Strategy (node-sharded, degree-grouped):
  - Nodes are sorted by in-degree globally and dealt round-robin to the 8
    cores, so every core sees the same (degree -> node count) profile after
    small dummy-node padding.  Each core owns all edges of its nodes, so no
    cross-core collectives are needed.
  - Per-edge work runs in a feature-on-partition layout [128 feat, edges]:
      h1 = A[dst] + B[src] + C[bond]   (pre-MLP layer 1, block-diagonal)
    A (dst side, own nodes) and C (5-row bond table) are applied via a
    host-built one-hot matmul on PE; B (src side, all nodes) is fetched with
    an SBUF-source transposed dma_gather and added via an identity matmul.
    m = relu(h1) @ BD(pre_w2) on PE.
  - Segment reductions (sum/min/max/sumsq) become fixed-stride innermost-
    axis tensor_reduce ops because each degree group has constant d.
  - Post-MLP, final linear, LayerNorm and the relu residual are per-node
    matmuls/elementwise in the same layout; output is transposed on PE and
    un-permuted on the host.
"""

import sys
import numpy as np

sys.path.insert(0, "/opt/trn_rl_repo")

import ml_dtypes

N, E, H, T, FIN, DEGBINS = 20000, 320000, 128, 4, 32, 128
NCORES = 8
CH = 1024          # edges per compute chunk
GIDX = 4096        # indices per dma_gather call (single_packet=False mode)
EPS_LN = 1e-5
ATILE = 123        # node columns per Acomb tile (+5 bond rows = 128)
NTOKP = 20096      # node table padded to 157*128

BF16 = ml_dtypes.bfloat16


# ----------------------------------------------------------------------------
# Planning: uniform per-core structure derived from the degree profile
# ----------------------------------------------------------------------------

class Plan:
    pass


def make_plan(edge_index):
    src = np.asarray(edge_index[0]).astype(np.int64)
    dst = np.asarray(edge_index[1]).astype(np.int64)
    deg = np.bincount(dst, minlength=N)

    order = np.argsort(deg, kind="stable")          # nodes by ascending degree
    core_nodes = [order[c::NCORES] for c in range(NCORES)]

    degrees = [int(d) for d in np.unique(deg)]
    n_d = {}
    for d in degrees:
        n_d[d] = max(int((deg[cn] == d).sum()) for cn in core_nodes)

    n_tot = sum(n_d.values())
    pad0 = (-n_tot) % 128
    if pad0:
        n_d[0] = n_d.get(0, 0) + pad0
        n_tot += pad0
    dlist = sorted(n_d.keys())

    col_deg = np.concatenate([np.full(n_d[d], d, np.int64) for d in dlist])
    assert len(col_deg) == n_tot and n_tot % 128 == 0

    # chunking (identical across cores)
    chunks = []
    c = 0
    while c < n_tot:
        e_used, c0 = 0, c
        while c < n_tot and e_used + col_deg[c] <= CH:
            e_used += col_deg[c]
            c += 1
        assert c > c0, "degree larger than chunk size"
        runs, eoff, rc = [], 0, c0
        while rc < c:
            d = int(col_deg[rc])
            rn = rc
            while rn < c and col_deg[rn] == d:
                rn += 1
            if d > 0:
                runs.append((int(rc), int(rn - rc), d, int(eoff)))
            eoff += (rn - rc) * d
            rc = rn
        chunks.append(dict(c0=int(c0), c1=int(c), nedge=int(e_used), runs=runs))
    while len(chunks) % (GIDX // CH):
        chunks.append(dict(c0=n_tot, c1=n_tot, nedge=0, runs=[]))
    nch = len(chunks)

    # A-tile matmul spans per chunk: (tile, j0, j1) over the chunk's CH slots
    for ch in chunks:
        spans, j = [], 0
        for col in range(ch["c0"], ch["c1"]):
            t, d = col // ATILE, int(col_deg[col])
            if d == 0:
                continue
            if spans and spans[-1][0] == t:
                spans[-1][2] += d
            else:
                spans.append([t, j, j + d])
            j += d
        if spans:
            spans[-1][2] = CH          # cover pad slots (one-hot zero there)
        else:
            spans = [[0, 0, CH]]
        ch["spans"] = [(int(a), int(b), int(cc)) for a, b, cc in spans]

    p = Plan()
    p.deg, p.core_nodes = deg, core_nodes
    p.n_d, p.dlist, p.n_tot, p.col_deg = n_d, dlist, n_tot, col_deg
    p.chunks, p.nch, p.ngrp = chunks, nch, nch
    p.zero_cols = int(n_d.get(0, 0))
    p.ntiles = (n_tot + ATILE - 1) // ATILE

    # per-core column -> global node id (-1 for dummy)
    p.col_node = []
    for cn in core_nodes:
        cols = np.full(n_tot, -1, np.int64)
        off = 0
        for d in dlist:
            mine = cn[deg[cn] == d]
            cols[off:off + len(mine)] = mine
            off += n_d[d]
        p.col_node.append(cols)

    eo = np.argsort(dst, kind="stable")
    starts = np.zeros(N + 1, np.int64)
    np.cumsum(np.bincount(dst, minlength=N), out=starts[1:])
    p.edge_order, p.edge_starts = eo, starts
    p.src, p.bond_holder = src, None
    return p


def pack_core_edges(p, c, src, bond):
    """Per-core padded edge arrays: src ids, bond ids, one-hot rows."""
    nch, n_tot = p.nch, p.n_tot
    EP = nch * CH
    srcv = np.zeros(EP, np.int64)
    bondv = np.zeros(EP, np.int64)
    arow = np.full(EP, -1, np.int64)    # 0..122 within tile, -1 = pad slot
    cols = p.col_node[c]
    eo, starts = p.edge_order, p.edge_starts
    for k, ch in enumerate(p.chunks):
        j = k * CH
        for col in range(ch["c0"], ch["c1"]):
            d = int(p.col_deg[col])
            if d == 0:
                continue
            node = cols[col]
            if node >= 0:
                eidx = eo[starts[node]:starts[node + 1]]
                srcv[j:j + d] = src[eidx]
                bondv[j:j + d] = bond[eidx]
            arow[j:j + d] = col % ATILE
            j += d
    return srcv, bondv, arow


def build_expdat(p, srcv, bondv, arow):
    EP = p.nch * CH
    exp = np.zeros((p.nch, 128, CH), BF16)
    k = np.arange(EP) // CH
    j = np.arange(EP) % CH
    real = arow >= 0
    exp[k[real], arow[real], j[real]] = 1
    exp[k[real], 123 + bondv[real], j[real]] = 1
    return exp


def build_srcidx(p, srcv):
    """[ngrp, 128, GIDX//16]; one gather call per group of GIDX edges."""
    ngrp = (p.nch * CH) // GIDX
    out = np.zeros((ngrp, 128, GIDX // 16), np.int16)
    i = np.arange(GIDX)
    v = srcv.reshape(ngrp, GIDX).astype(np.int16)
    for rep in range(8):
        out[:, rep * 16 + (i % 16), i // 16] = v[:, i]
    return out


# ----------------------------------------------------------------------------
# Weight packing (host)
# ----------------------------------------------------------------------------

def blockdiag(ws):
    """ws: [T, f, g] -> [T*f, T*g] block-diagonal."""
    ws = np.asarray(ws)
    t, f, g = ws.shape
    out = np.zeros((t * f, t * g), np.float32)
    for i in range(t):
        out[i * f:(i + 1) * f, i * g:(i + 1) * g] = ws[i]
    return out


def pack_weights(ins, p):
    w = {}
    pre_w1 = np.asarray(ins["pre_w1"], np.float32)      # [T, 96, 32]
    w["wa"] = np.ascontiguousarray(blockdiag(pre_w1[:, 0:FIN]))            # f32
    w["wb"] = blockdiag(pre_w1[:, FIN:2 * FIN]).astype(BF16)
    table5 = (np.asarray(ins["bond_emb"], np.float32) @ np.asarray(ins["enc_w"], np.float32)
              + np.asarray(ins["enc_b"], np.float32))                       # [5, 32]
    wc = pre_w1[:, 2 * FIN:3 * FIN]                                        # [T, 32, 32]
    ctab = np.zeros((5, H), np.float32)
    b1 = np.asarray(ins["pre_b1"], np.float32)                             # [T, 32]
    for k in range(5):
        for t in range(T):
            ctab[k, t * FIN:(t + 1) * FIN] = table5[k] @ wc[t] + b1[t]
    w["ctab"] = ctab.astype(BF16)
    w["w2"] = blockdiag(np.asarray(ins["pre_w2"], np.float32)).astype(BF16)

    post_w1 = np.asarray(ins["post_w1"], np.float32)                       # [T, 512, 32]
    blocks = []
    rowsel = [0] + [1 + a for a in range(5)] + [6 + a for a in range(5)] + [11 + a for a in range(5)]
    for r in rowsel:
        blocks.append(blockdiag(post_w1[:, 32 * r:32 * (r + 1)]))
    w["w1post"] = np.stack(blocks).astype(BF16)                            # [16,128,128]
    w["w1postf"] = np.ascontiguousarray(
        np.stack([blocks[i] for i in (1, 2, 6, 7, 11, 12)]))               # f32 sum/mean
    w["w2post"] = blockdiag(np.asarray(ins["post_w2"], np.float32)).astype(BF16)
    w["wlin"] = np.ascontiguousarray(np.asarray(ins["lin_w"], np.float32))

    biases = np.zeros((128, 5), np.float32)
    biases[:, 0] = np.asarray(ins["post_b1"], np.float32).reshape(-1)
    biases[:, 1] = np.asarray(ins["post_b2"], np.float32).reshape(-1)
    biases[:, 2] = np.asarray(ins["lin_b"], np.float32)
    biases[:, 3] = np.asarray(ins["ln_g"], np.float32)
    biases[:, 4] = np.asarray(ins["ln_b"], np.float32)
    w["biases"] = biases

    w["identb"] = np.eye(128, dtype=BF16)
    w["identf"] = np.eye(128, dtype=np.float32)
    w["onescol"] = np.ones((128, 1), np.float32)
    w["onesrow"] = np.ones((1, 128), np.float32)

    # per-node scaler rows and the pre_b2 correction tensor
    mol_deg = np.asarray(ins["mol_deg"], np.float64)
    bins = np.arange(DEGBINS, dtype=np.float64)
    avg_log = float((np.log(bins + 1.0) * mol_deg).sum() / mol_deg.sum())

    d = p.col_deg.astype(np.float64)
    d1 = np.maximum(d, 1.0)
    logd = np.log(d1 + 1.0)
    invd = (1.0 / d1).astype(np.float32)
    amp = (logd / avg_log).astype(np.float32)
    att = (avg_log / logd).astype(np.float32)
    w["invd_bc"] = np.ascontiguousarray(np.broadcast_to(invd, (128, p.n_tot)))
    w["amp_bc"] = np.ascontiguousarray(np.broadcast_to(amp, (128, p.n_tot))).astype(BF16)
    w["att_bc"] = np.ascontiguousarray(np.broadcast_to(att, (128, p.n_tot))).astype(BF16)

    b2 = np.asarray(ins["pre_b2"], np.float32).reshape(-1)                  # [128]
    # correction for reducing m without its bias: mean/min/max shift by b2,
    # sum by d*b2; folded through the post-MLP first-layer weights.
    WT = w["w1post"].astype(np.float32)          # [16,128,128] lhsT = [f_in, f_out]
    def grp(i0):  # sum of (W.T @ b2) for blocks mean,min,max at i0+1.. ; sum block at i0
        csum = WT[i0].T @ b2
        cmmm = sum(WT[i0 + a].T @ b2 for a in (1, 2, 3))
        return csum, cmmm
    cs_id, cm_id = grp(1)
    cs_am, cm_am = grp(6)
    cs_at, cm_at = grp(11)
    dcol = d.astype(np.float32)
    corr = (cm_id[:, None] + np.outer(cs_id, dcol)
            + amp[None, :] * (cm_am[:, None] + np.outer(cs_am, dcol))
            + att[None, :] * (cm_at[:, None] + np.outer(cs_at, dcol)))
    w["corr"] = np.ascontiguousarray(corr).astype(BF16)
    return w


# ----------------------------------------------------------------------------
# Device program
# ----------------------------------------------------------------------------

def build_program(p, phase="full"):
    import concourse.bass as bass
    import concourse.tile as tile
    from concourse import bacc, mybir

    f32 = mybir.dt.float32
    bf16 = mybir.dt.bfloat16
    i16 = mybir.dt.int16
    AX = mybir.AxisListType.X
    OP = mybir.AluOpType
    AF = mybir.ActivationFunctionType

    n_tot, nch, ntiles = p.n_tot, p.nch, p.ntiles

    nc = bacc.Bacc("TRN2", target_bir_lowering=False, debug=False)

    d_exp = nc.dram_tensor("expdat", [nch, 128, CH], bf16, kind="ExternalInput")
    ngrp = (nch * CH) // GIDX
    d_idx = nc.dram_tensor("srcidx", [ngrp, 128, GIDX // 16], i16,
                           kind="ExternalInput")
    d_xown = nc.dram_tensor("xown", [128, n_tot], f32, kind="ExternalInput")
    d_xfull = nc.dram_tensor("xfull", [128, NTOKP], bf16, kind="ExternalInput")
    d_wa = nc.dram_tensor("wa", [128, 128], f32, kind="ExternalInput")
    d_wb = nc.dram_tensor("wb", [128, 128], bf16, kind="ExternalInput")
    d_ctab = nc.dram_tensor("ctab", [5, 128], bf16, kind="ExternalInput")
    d_w2 = nc.dram_tensor("w2", [128, 128], bf16, kind="ExternalInput")
    d_w1post = nc.dram_tensor("w1post", [16, 128, 128], bf16, kind="ExternalInput")
    d_w1postf = nc.dram_tensor("w1postf", [6, 128, 128], f32, kind="ExternalInput")
    d_w2post = nc.dram_tensor("w2post", [128, 128], bf16, kind="ExternalInput")
    d_wlin = nc.dram_tensor("wlin", [128, 128], f32, kind="ExternalInput")
    d_bias = nc.dram_tensor("biases", [128, 5], f32, kind="ExternalInput")
    d_idb = nc.dram_tensor("identb", [128, 128], bf16, kind="ExternalInput")
    d_idf = nc.dram_tensor("identf", [128, 128], f32, kind="ExternalInput")
    d_onescol = nc.dram_tensor("onescol", [128, 1], f32, kind="ExternalInput")
    d_onesrow = nc.dram_tensor("onesrow", [1, 128], f32, kind="ExternalInput")
    d_invd = nc.dram_tensor("invd_bc", [128, n_tot], f32, kind="ExternalInput")
    d_amp = nc.dram_tensor("amp_bc", [128, n_tot], bf16, kind="ExternalInput")
    d_att = nc.dram_tensor("att_bc", [128, n_tot], bf16, kind="ExternalInput")
    d_corr = nc.dram_tensor("corr", [128, n_tot], bf16, kind="ExternalInput")
    d_out = nc.dram_tensor("out", [n_tot, 128], f32, kind="ExternalOutput")

    with tile.TileContext(nc) as tc:
        with tc.tile_pool(name="persist", bufs=1) as per:
            # --- persistent SBUF tensors -------------------------------------
            # Edge-critical loads first: the gather table, then the pre-MLP
            # weights; everything else is deferred into the edge loop so the
            # DMA queue doesn't delay the first gather.
            xtok = per.tile([128, NTOKP], bf16)
            nc.sync.dma_start(xtok[:], d_xfull[:])
            xown = per.tile([128, n_tot], f32)
            nc.sync.dma_start(xown[:], d_xown[:])
            wa = per.tile([128, 128], f32); nc.sync.dma_start(wa[:], d_wa[:])
            wb = per.tile([128, 128], bf16); nc.sync.dma_start(wb[:], d_wb[:])
            w2 = per.tile([128, 128], bf16); nc.sync.dma_start(w2[:], d_w2[:])
            w1post = per.tile([128, 16, 128], bf16)
            w1postf = per.tile([128, 6, 128], f32)
            w2post = per.tile([128, 128], bf16)
            wlin = per.tile([128, 128], f32)
            biases = per.tile([128, 5], f32)
            identb = per.tile([128, 128], bf16)
            identf = per.tile([128, 128], f32)
            onescol = per.tile([128, 1], f32)
            onesrow = per.tile([1, 128], f32)
            invd_bc = per.tile([128, n_tot], f32)
            amp_bc = per.tile([128, n_tot], bf16)
            att_bc = per.tile([128, n_tot], bf16)
            corr = per.tile([128, n_tot], bf16)
            eps_col = per.tile([128, 1], f32)
            nc.gpsimd.memset(eps_col[:], EPS_LN)

            def load_deferred(step=None):
                def s0():
                    for jj in range(16):
                        nc.sync.dma_start(w1post[:, jj, :], d_w1post[jj, :, :])
                    nc.sync.dma_start(biases[:], d_bias[:])
                    nc.sync.dma_start(identb[:], d_idb[:])
                def s1():
                    for jj in range(6):
                        nc.sync.dma_start(w1postf[:, jj, :], d_w1postf[jj, :, :])
                    nc.sync.dma_start(w2post[:], d_w2post[:])
                    nc.sync.dma_start(wlin[:], d_wlin[:])
                    nc.sync.dma_start(onescol[:], d_onescol[:])
                    nc.sync.dma_start(onesrow[:], d_onesrow[:])
                    nc.sync.dma_start(identf[:], d_idf[:])
                def s2():
                    nc.sync.dma_start(invd_bc[:], d_invd[:])
                def s3():
                    nc.sync.dma_start(amp_bc[:], d_amp[:])
                def s4():
                    nc.sync.dma_start(att_bc[:], d_att[:])
                    nc.sync.dma_start(corr[:], d_corr[:])
                steps = [s0, s1, s2, s3, s4]
                if step is None:
                    for s in steps:
                        s()
                else:
                    steps[step]()

            acomb = per.tile([128, ntiles, 128], bf16)
            nc.vector.memset(acomb[:], 0.0)

            s_sum = per.tile([128, n_tot], f32)
            s_sumsq = per.tile([128, n_tot], f32)
            s_min = per.tile([128, n_tot], bf16)
            s_max = per.tile([128, n_tot], bf16)
            x_bf = per.tile([128, n_tot], bf16)
            nc.scalar.activation(x_bf[:], xown[:], mybir.ActivationFunctionType.Copy)
            lbuf = per.tile([128, n_tot], f32)
            row_mu = per.tile([1, n_tot], f32)
            row_sq = per.tile([1, n_tot], f32)

            # --- setup: Acomb ------------------------------------------------
            with tc.tile_pool(name="su_ps", bufs=2, space="PSUM") as sps:
                for t in range(ntiles):
                    nt = min(ATILE, n_tot - t * ATILE)
                    aps = sps.tile([128, 128], f32, tag="aps")
                    nc.tensor.matmul(aps[0:nt, :], xown[:, t * ATILE:t * ATILE + nt],
                                     wa[:], start=True, stop=True)
                    nc.scalar.activation(acomb[0:nt, t, :], aps[0:nt, :], AF.Copy)
                    nc.sync.dma_start(acomb[ATILE:128, t, :], d_ctab[:, :])

            if phase == "setup":
                load_deferred()
            if p.zero_cols:
                z = p.zero_cols
                nc.vector.memset(s_sum[:, 0:z], 0.0)
                nc.vector.memset(s_sumsq[:, 0:z], 0.0)
                nc.vector.memset(s_min[:, 0:z], 0.0)
                nc.vector.memset(s_max[:, 0:z], 0.0)

            # --- edge phase (with post-MLP layer-1 interleaved) ------------
            if phase != "setup":
              # col-chunk c is ready after the last edge chunk touching its cols
              ready = {}
              if phase == "full":
                  for cc in range(n_tot // 512):
                      last = 0
                      for k2, ch2 in enumerate(p.chunks):
                          if ch2["c0"] < 512 * (cc + 1) and ch2["runs"]:
                              last = k2
                      ready.setdefault(last, []).append(cc)
              with (
                tc.tile_pool(name="eg_h1", bufs=2, space="PSUM") as ph1,
                tc.tile_pool(name="eg_m", bufs=2, space="PSUM") as pm,
                tc.tile_pool(name="eg_sb", bufs=2) as esb,
                tc.tile_pool(name="eg_gt", bufs=2) as egt,
                tc.tile_pool(name="eg_pa", bufs=1, space="PSUM") as ppa,
                tc.tile_pool(name="eg_pasb", bufs=2) as pasb,
              ):
                first_ready = min(ready.keys()) if ready else 3
                lodeadline = max(1, min(3, first_ready - 2))
                for k in range(nch):
                    if phase != "setup":
                        # staggered, but every step lands before first_ready
                        for st, kk_at in ((0, lodeadline), (1, lodeadline),
                                          (2, min(lodeadline + 1, first_ready)),
                                          (3, min(lodeadline + 1, first_ready)),
                                          (4, min(lodeadline + 2, first_ready))):
                            if k == kk_at:
                                load_deferred(st)
                    kk = k % (GIDX // CH)
                    if kk == 0:
                        gidx = egt.tile([128, GIDX // 16], i16, tag="gidx")
                        nc.sync.dma_start(gidx[:], d_idx[k // (GIDX // CH), :, :])
                        bg = egt.tile([128, 1, GIDX], bf16, tag="bg")
                        nc.gpsimd.dma_gather(
                            bg[:], xtok[:], gidx[:], GIDX, GIDX, 128,
                            transpose=True, sbuf_tokens_per_rank=128,
                            sbuf_free_dim_per_rank=256, single_packet=False,
                        )
                    if phase == "gather":
                        continue
                    ch = p.chunks[k]
                    if ch["runs"]:
                        expt = esb.tile([128, CH], bf16, tag="expt")
                        nc.sync.dma_start(expt[:], d_exp[k, :, :])
                        h1 = ph1.tile([128, CH], f32, tag="h1", bufs=1)
                        for a in range(0, CH, 512):
                            b = a + 512
                            first = True
                            for (t, j0, j1) in ch["spans"]:
                                lo, hi = max(j0, a), min(j1, b)
                                if lo >= hi:
                                    continue
                                nc.tensor.matmul(h1[:, lo:hi], acomb[:, t, :],
                                                 expt[:, lo:hi], start=first,
                                                 stop=False,
                                                 skip_group_check=True)
                                first = False
                            nc.tensor.matmul(h1[:, a:b], wb[:],
                                             bg[:, 0, kk * CH + a:kk * CH + b],
                                             start=first, stop=True,
                                             skip_group_check=True)
                        r = esb.tile([128, CH], bf16, tag="r")
                        nc.scalar.activation(r[:], h1[:], AF.Relu)
                        mps = pm.tile([128, CH], f32, tag="mps", bufs=1)
                        for a in range(0, CH, 512):
                            nc.tensor.matmul(mps[:, a:a + 512], w2[:],
                                             r[:, a:a + 512], start=True,
                                             stop=True)
                        if phase in ("edge",):
                            continue
                        msq = esb.tile([128, CH], f32, tag="msq", bufs=1)
                        nc.scalar.activation(msq[:], mps[:], AF.Square)
                        mbf = esb.tile([128, CH], bf16, tag="mbf", bufs=1)
                        nc.scalar.activation(mbf[:], mps[:], AF.Copy)
                        for (col0, ncols, d, eoff) in ch["runs"]:
                            vm = mbf[:, eoff:eoff + ncols * d].rearrange(
                                "p (n d) -> p n d", d=d)
                            vq = msq[:, eoff:eoff + ncols * d].rearrange(
                                "p (n d) -> p n d", d=d)
                            nc.vector.tensor_reduce(
                                s_sum[:, col0:col0 + ncols], vm, axis=AX, op=OP.add)
                            nc.vector.tensor_reduce(
                                s_min[:, col0:col0 + ncols], vm, axis=AX, op=OP.min)
                            nc.vector.tensor_reduce(
                                s_max[:, col0:col0 + ncols], vm, axis=AX, op=OP.max)
                            nc.vector.tensor_reduce(
                                s_sumsq[:, col0:col0 + ncols], vq, axis=AX, op=OP.add)
                    # interleaved post-MLP layer 1 for completed column chunks
                    for cc in ready.get(k, []):
                        a, b = 512 * cc, 512 * (cc + 1)
                        s_mean = pasb.tile([128, 512], f32, tag="tmean")
                        nc.vector.tensor_tensor(s_mean[:], s_sum[:, a:b],
                                                invd_bc[:, a:b], op=OP.mult)
                        var = pasb.tile([128, 512], f32, tag="ta")
                        nc.vector.tensor_tensor(var[:], s_sumsq[:, a:b],
                                                invd_bc[:, a:b], op=OP.mult)
                        m2 = pasb.tile([128, 512], f32, tag="tb")
                        nc.vector.tensor_tensor(m2[:], s_mean[:],
                                                s_mean[:], op=OP.mult)
                        nc.vector.tensor_tensor(var[:], var[:], m2[:],
                                                op=OP.subtract)
                        vclamp = pasb.tile([128, 512], f32, tag="tvc")
                        nc.scalar.activation(vclamp[:], var[:], AF.Relu)
                        tstd = pasb.tile([128, 512], bf16, tag="tstd")
                        nc.scalar.activation(tstd[:], vclamp[:], AF.Sqrt,
                                             bias=eps_col[:])
                        uid = ppa.tile([128, 512], f32, tag="uid")
                        uam = ppa.tile([128, 512], f32, tag="uam")
                        uat = ppa.tile([128, 512], f32, tag="uat")
                        nc.tensor.matmul(uid[:], w1post[:, 0, :], x_bf[:, a:b],
                                         start=True, stop=False)
                        nc.tensor.matmul(uid[:], w1postf[:, 0, :], s_sum[:, a:b],
                                         start=False, stop=False)
                        nc.tensor.matmul(uid[:], w1postf[:, 1, :], s_mean[:],
                                         start=False, stop=False)
                        nc.tensor.matmul(uid[:], w1post[:, 3, :], s_min[:, a:b],
                                         start=False, stop=False)
                        nc.tensor.matmul(uid[:], w1post[:, 4, :], s_max[:, a:b],
                                         start=False, stop=False)
                        nc.tensor.matmul(uid[:], w1post[:, 5, :], tstd[:],
                                         start=False, stop=False)
                        nc.tensor.matmul(uid[:], identb[:], corr[:, a:b],
                                         start=False, stop=True)
                        nc.tensor.matmul(uam[:], w1postf[:, 2, :], s_sum[:, a:b],
                                         start=True, stop=False)
                        nc.tensor.matmul(uam[:], w1postf[:, 3, :], s_mean[:],
                                         start=False, stop=False)
                        nc.tensor.matmul(uam[:], w1post[:, 8, :], s_min[:, a:b],
                                         start=False, stop=False)
                        nc.tensor.matmul(uam[:], w1post[:, 9, :], s_max[:, a:b],
                                         start=False, stop=False)
                        nc.tensor.matmul(uam[:], w1post[:, 10, :], tstd[:],
                                         start=False, stop=True)
                        nc.tensor.matmul(uat[:], w1postf[:, 4, :], s_sum[:, a:b],
                                         start=True, stop=False)
                        nc.tensor.matmul(uat[:], w1postf[:, 5, :], s_mean[:],
                                         start=False, stop=False)
                        nc.tensor.matmul(uat[:], w1post[:, 13, :], s_min[:, a:b],
                                         start=False, stop=False)
                        nc.tensor.matmul(uat[:], w1post[:, 14, :], s_max[:, a:b],
                                         start=False, stop=False)
                        nc.tensor.matmul(uat[:], w1post[:, 15, :], tstd[:],
                                         start=False, stop=True)
                        t1 = pasb.tile([128, 512], f32, tag="ta")
                        nc.vector.tensor_tensor(t1[:], uam[:], amp_bc[:, a:b],
                                                op=OP.mult)
                        t2 = pasb.tile([128, 512], f32, tag="tb")
                        nc.vector.tensor_tensor(t2[:], uat[:], att_bc[:, a:b],
                                                op=OP.mult)
                        nc.vector.tensor_tensor(t1[:], t1[:], t2[:], op=OP.add)
                        o1 = pasb.tile([128, 512], bf16, tag="o1")
                        nc.vector.tensor_tensor(o1[:], uid[:], t1[:], op=OP.add)
                        # pass B inline: layer 2, final linear, LN statistics
                        r1 = pasb.tile([128, 512], bf16, tag="tr")
                        nc.scalar.activation(r1[:], o1[:], AF.Relu,
                                             bias=biases[:, 0:1])
                        u2 = ppa.tile([128, 512], f32, tag="uid")
                        nc.tensor.matmul(u2[:], w2post[:], r1[:],
                                         start=True, stop=True)
                        s2 = pasb.tile([128, 512], f32, tag="ta")
                        nc.scalar.activation(s2[:], u2[:], AF.Identity,
                                             bias=biases[:, 1:2])
                        lp = ppa.tile([128, 512], f32, tag="uam")
                        nc.tensor.matmul(lp[:], wlin[:], s2[:],
                                         start=True, stop=True)
                        nc.scalar.activation(lbuf[:, a:b], lp[:], AF.Identity,
                                             bias=biases[:, 2:3])
                        lsq = pasb.tile([128, 512], f32, tag="tb")
                        nc.vector.tensor_tensor(lsq[:], lbuf[:, a:b],
                                                lbuf[:, a:b], op=OP.mult)
                        mu_p = ppa.tile([1, 512], f32, tag="uat")
                        nc.tensor.matmul(mu_p[:], onescol[:], lbuf[:, a:b],
                                         start=True, stop=True)
                        nc.scalar.activation(row_mu[:, a:b], mu_p[:], AF.Copy,
                                             scale=1.0 / 128.0)
                        sq_p = ppa.tile([1, 512], f32, tag="uat")
                        nc.tensor.matmul(sq_p[:], onescol[:], lsq[:],
                                         start=True, stop=True)
                        nc.scalar.activation(row_sq[:, a:b], sq_p[:], AF.Copy,
                                             scale=1.0 / 128.0)

            # --- post tail: layer2, linear, LN, residual, transpose ----------
            if phase == "full":
              with tc.tile_pool(name="po_sb", bufs=2) as psb:
                outbuf = psb.tile([128, n_tot], f32, bufs=1, tag="outbuf")
                row_rs = psb.tile([1, n_tot], f32, bufs=1, tag="row_rs")

                nc.vector.tensor_tensor(row_rs[:], row_mu[:], row_mu[:],
                                        op=OP.mult)
                nc.vector.tensor_tensor(row_rs[:], row_sq[:], row_rs[:],
                                        op=OP.subtract)
                nc.scalar.activation(row_sq[:], row_rs[:], AF.Sqrt,
                                     bias=eps_col[0:1, :])
                nc.vector.reciprocal(row_rs[:], row_sq[:])

                with tc.tile_pool(name="po_psD", bufs=2, space="PSUM") as pps:
                    for a in range(0, n_tot, 512):
                        b = a + 512
                        mub = pps.tile([128, 512], f32, tag="mub")
                        nc.tensor.matmul(mub[:], onesrow[:], row_mu[:, a:b],
                                         start=True, stop=True)
                        rsb = pps.tile([128, 512], f32, tag="rsb")
                        nc.tensor.matmul(rsb[:], onesrow[:], row_rs[:, a:b],
                                         start=True, stop=True)
                        tt = psb.tile([128, 512], f32, tag="ta")
                        nc.vector.tensor_tensor(tt[:], lbuf[:, a:b], mub[:],
                                                op=OP.subtract)
                        nc.vector.tensor_tensor(tt[:], tt[:], rsb[:], op=OP.mult)
                        fin = psb.tile([128, 512], f32, tag="tb")
                        nc.scalar.activation(fin[:], tt[:], AF.Relu,
                                             bias=biases[:, 4:5],
                                             scale=biases[:, 3:4])
                        nc.vector.tensor_tensor(outbuf[:, a:b], fin[:],
                                                xown[:, a:b], op=OP.add)
                    for t in range(n_tot // 128):
                        tp = pps.tile([128, 128], f32, tag="tp")
                        nc.tensor.transpose(tp[:], outbuf[:, t * 128:(t + 1) * 128],
                                            identf[:])
                        orow = psb.tile([128, 128], f32, tag="orow")
                        nc.scalar.activation(orow[:], tp[:], AF.Copy)
                        nc.sync.dma_start(
                            d_out.rearrange("(t p) f -> t p f", p=128)[t, :, :],
                            orow[:])

    nc.compile()
    return nc


# ----------------------------------------------------------------------------
# Entry point
# ----------------------------------------------------------------------------

_CACHE = {}
LAST_EXEC_NS = None
LAST_TRACE = None


def kernel(**inputs):
    from concourse.bass_utils import run_bass_kernel_spmd

    atom_x = np.asarray(inputs["atom_x"], np.float32)
    bond_x = np.asarray(inputs["bond_x"]).astype(np.int64)
    edge_index = np.asarray(inputs["edge_index"])
    src = edge_index[0].astype(np.int64)

    p = make_plan(edge_index)
    w = pack_weights(inputs, p)

    xtok = np.zeros((128, NTOKP), BF16)
    xb = atom_x.astype(BF16)
    ii = np.arange(N)
    xtok_view = xtok.reshape(128, NTOKP // 128, 128)
    xtok_view[ii % 128, ii // 128, :] = xb

    in_maps = []
    for c in range(NCORES):
        srcv, bondv, arow = pack_core_edges(p, c, src, bond_x)
        exp = build_expdat(p, srcv, bondv, arow)
        sidx = build_srcidx(p, srcv)
        cols = p.col_node[c]
        xo = np.zeros((128, p.n_tot), np.float32)
        real = cols >= 0
        xo[:, real] = atom_x[cols[real]].T
        m = dict(expdat=exp, srcidx=sidx, xown=xo, xfull=xtok)
        m.update({k: w[k] for k in (
            "wa", "wb", "ctab", "w2", "w1post", "w1postf", "w2post", "wlin",
            "biases", "identb", "identf", "onescol", "onesrow", "invd_bc",
            "amp_bc", "att_bc", "corr")})
        in_maps.append(m)

    key = (p.n_tot, p.nch, str([c["runs"] for c in p.chunks]))
    if key not in _CACHE:
        _CACHE[key] = build_program(p)
    nc = _CACHE[key]

    res = run_bass_kernel_spmd(nc, in_maps, core_ids=list(range(NCORES)))
    global LAST_EXEC_NS
    LAST_EXEC_NS = res.exec_time_ns

    out = np.empty((N, H), np.float32)
    for c in range(NCORES):
        cols = p.col_node[c]
        real = cols >= 0
        out[cols[real]] = res.results[c]["out"][real]
    return out

